# revision 48
# baseline (speedup 1.0000x reference)
"""Single-head causal attention on 8 TRN2 NeuronCores (Bass/Tile).

Problem: x[B=4,T=4096,E=1024] fp32; Wq/Wk/Wv [E,64]. out = softmax(causal(QK^T/8)) V.

Sharding: core i = (batch b=i//2, parity p=i%2). Each core computes the output
rows for the 256-token blocks of batch b with block index ≡ p (mod 2); one
uniform SPMD program, all per-core variation is input data.

v2 datapath (all-bf16 matmuls, 512-query spans):
  K^T,V^T projected packed in bf16 (PSUM fp32 acc over 8 E-chunks); V^T
  transposed to V-natural via PE in bf16. Q^T projected per 512-token span.
  Scores S^T[k,q] as [128,512] tiles; exp on ACT over paired 2-bank PSUM
  reads [128,1024] -> bf16 P (no max subtraction; |score*scale| <= ~3.6).
  Causal masks via (iota >= D) * P on DVE: own-parity tails use iota1=c-ch
  with constant D in {0,128,256,384}; other-parity tails use iota2=c with
  per-core D from dtab. P^T @ [V|1] accumulates O^T + softmax denominator in
  one PSUM group per span. Epilogue: reciprocal of den row, GpSimd
  partition-broadcast, DVE multiply; out stored transposed [H, T/2] (host
  gather transposes). KV projection of later tiles is interleaved between
  attention pairs to keep the PE busy while ACT drains.
"""

import os
import numpy as np

import concourse.bass as bass
import concourse.tile as tile
from concourse import bacc, bass_utils, mybir
from concourse.masks import make_identity

F32 = mybir.dt.float32
BF16 = mybir.dt.bfloat16
AF = mybir.ActivationFunctionType
ALU = mybir.AluOpType

B, T_FULL, E, H = 4, 4096, 1024, 64
NCORES = 8
SCALE = float(H) ** -0.5


def build_program(T):
    EC = E // 128            # 8 E-chunks
    NT = T // 512            # 8 x^T tiles (0-3 own tokens, 4-7 other)
    NSP = T // 1024          # 4 spans of 512 own queries
    NKT = T // 128           # 32 total 128-key tiles
    KO = NKT // 2            # 16 own k-tiles (kt col offset T//2 for other)

    nc = bacc.Bacc(
        "TRN2", target_bir_lowering=False, debug=False, num_devices=NCORES
    )
    # x^T pre-arranged by host as [p, tile, chunk, col] so each per-partition
    # DMA run is contiguous (8KB descriptors instead of 1KB).
    xt_d = nc.dram_tensor("xt", [128, (T // 512) * EC * 512], BF16, kind="ExternalInput")
    wkv_d = nc.dram_tensor("wkv", [128, EC * 2 * H], BF16, kind="ExternalInput")
    wq_d = nc.dram_tensor("wq", [128, EC * H], BF16, kind="ExternalInput")
    mask_d = nc.dram_tensor("masks", [128, 8, 512], BF16, kind="ExternalInput")
    out_d = nc.dram_tensor("out", [H, T // 2], F32, kind="ExternalOutput")

    with tile.TileContext(nc) as tc:
        with (
            tc.tile_pool(name="persist", bufs=1) as pp,
            tc.tile_pool(name="stage", bufs=3) as sp,
            tc.tile_pool(name="ppool", bufs=4) as ptp,
            tc.tile_pool(name="rdp", bufs=2) as rdp,
            tc.tile_pool(name="rbp", bufs=2) as rbp,
            tc.tile_pool(name="obp", bufs=2) as obp,
        ):
            # ---- persistent SBUF ----
            xt = [pp.tile([128, EC, 512], BF16, tag=f"xt{t}", name=f"xt{t}") for t in range(NT)]
            kt = pp.tile([64, T], BF16, tag="kt")
            vb = pp.tile([128, NKT, H + 1], BF16, tag="vb")
            qt = pp.tile([64, NSP, 512], BF16, tag="qt")
            wkv = pp.tile([128, EC, 2 * H], BF16, tag="wkv")
            wq = pp.tile([128, EC, H], BF16, tag="wq")
            masks = pp.tile([128, 8, 512], BF16, tag="masks")
            identb = pp.tile([128, 128], BF16, tag="identb")
            identf = pp.tile([H + 1, H + 1], F32, tag="identf")

            # ---- constants / small inputs ----
            nc.sync.dma_start(wkv, wkv_d.ap().rearrange("p (c m) -> p c m", c=EC))
            nc.sync.dma_start(wq, wq_d.ap().rearrange("p (c m) -> p c m", c=EC))
            make_identity(nc, identb)
            make_identity(nc, identf)
            nc.vector.memset(
                vb[:, :, H : H + 1].bitcast(mybir.dt.uint16), 0x3F80
            )

            # ---- stream x^T in program need order; first tile finer-grained ----
            xsrc = xt_d.ap().rearrange("p (n c t) -> p n c t", n=NT, c=EC)
            for c in range(0, 8, 2):
                nc.sync.dma_start(xt[0][:, c : c + 2, :], xsrc[:, 0, c : c + 2, :])
            for t in (1, NSP):
                nc.sync.dma_start(xt[t][:, 0:4, :], xsrc[:, t, 0:4, :])
                nc.sync.dma_start(xt[t][:, 4:8, :], xsrc[:, t, 4:8, :])
            nc.sync.dma_start(masks, mask_d.ap())
            for t in (NSP + 1, 2, NSP + 2, 3, NSP + 3):
                nc.sync.dma_start(xt[t][:, 0:4, :], xsrc[:, t, 0:4, :])
                nc.sync.dma_start(xt[t][:, 4:8, :], xsrc[:, t, 4:8, :])

            with (
                tc.tile_pool(name="spsum", bufs=2, space="PSUM") as ssp,
                tc.tile_pool(name="opsum", bufs=2, space="PSUM") as otp,
                tc.tile_pool(name="miscpsum", bufs=2, space="PSUM") as mp,
            ):
                def kv_proj(t):
                    kvfull = ssp.tile([128, 1024], F32, tag="s", name="kvfull")
                    acc = kvfull[:, 0:512]
                    for c in range(EC):
                        nc.tensor.matmul(
                            acc,
                            wkv[:, c, :],
                            xt[t][:, c, :],
                            start=(c == 0),
                            stop=(c == EC - 1),
                        )
                    kvs = sp.tile([128, 512], BF16, tag="kvs")
                    nc.vector.tensor_copy(kvs, acc)
                    nc.vector.tensor_copy(
                        kt[:, 512 * t : 512 * (t + 1)], kvs[0:64, :]
                    )
                    vg = mp.tile([128, 4, H], BF16, tag="vg")
                    for j in range(4):
                        nc.tensor.transpose(
                            vg[:, j, :],
                            kvs[64:128, 128 * j : 128 * (j + 1)],
                            identb[64:128, 64:128],
                        )
                    nc.vector.tensor_copy(vb[:, 4 * t : 4 * t + 4, 0:H], vg)

                def q_proj(s):
                    qfull = ssp.tile([128, 1024], F32, tag="s", name="qfull")
                    qacc = qfull[0:64, 0:512]
                    for c in range(EC):
                        nc.tensor.matmul(
                            qacc,
                            wq[:, c, :],
                            xt[s][:, c, :],
                            start=(c == 0),
                            stop=(c == EC - 1),
                        )
                    nc.vector.tensor_copy(qt[:, s, :], qacc)

                def span_pairs(s):
                    # k-tile list: (region_base_col, j, mask_col)
                    nk = 4 * s + 4
                    tiles = []
                    for j in range(nk):
                        m = (j - 4 * s) if j >= 4 * s else None
                        tiles.append((0, j, m))
                    for j in range(nk):
                        m = (4 + j - 4 * s) if j >= 4 * s else None
                        tiles.append((T // 2, j, m))
                    return [tiles[i : i + 2] for i in range(0, len(tiles), 2)]

                def do_pair(s, ot, pair, first, last):
                    st = ssp.tile([128, 1024], F32, tag="s", name="st")
                    for h, (base, j, m) in enumerate(pair):
                        kc = base + 128 * j
                        nc.tensor.matmul(
                            st[:, 512 * h : 512 * (h + 1)],
                            kt[:, kc : kc + 128],
                            qt[:, s, :],
                            start=True,
                            stop=True,
                        )
                    pt = ptp.tile([128, 1024], BF16, tag="pt")
                    nc.scalar.activation(pt, st, AF.Exp, scale=SCALE)
                    if pair[0][2] is not None or pair[1][2] is not None:
                        ptm = ptp.tile([128, 1024], BF16, tag="ptm")
                        for h, (base, j, m) in enumerate(pair):
                            nc.vector.scalar_tensor_tensor(
                                ptm[:, 512 * h : 512 * (h + 1)],
                                pt[:, 512 * h : 512 * (h + 1)],
                                1.0,
                                masks[:, m, :],
                                ALU.mult,
                                ALU.mult,
                            )
                        pt = ptm
                    for h, (base, j, m) in enumerate(pair):
                        vi = j if base == 0 else KO + j
                        nc.tensor.matmul(
                            ot,
                            vb[:, vi, :],
                            pt[:, 512 * h : 512 * (h + 1)],
                            start=(first and h == 0),
                            stop=(last and h == 1),
                        )

                def attention2(sa, sb, inject, epi_a):
                    """Interleave two spans' attention; span sa paced to
                    finish ~3 rounds early so its epilogue (epi_a) overlaps
                    span sb's ACT-bound tail. inject[i] thunks run at round
                    i."""
                    pa, pb = span_pairs(sa), span_pairs(sb)
                    na, nb = len(pa), len(pb)
                    ota = otp.tile([H + 1, 512], F32, tag="ot", name="ota")
                    otb = otp.tile([H + 1, 512], F32, tag="ot", name="otb")
                    ka = 0
                    done_a = False
                    for i in range(nb):
                        for fn in inject.get(i, []):
                            fn()
                        want = min(na, ((i + 1) * na) // max(1, nb - 3))
                        while ka < want:
                            do_pair(sa, ota, pa[ka], ka == 0, ka == na - 1)
                            ka += 1
                        if ka == na and not done_a:
                            done_a = True
                            epi_a(ota)
                        do_pair(sb, otb, pb[i], i == 0, i == nb - 1)
                    return otb
                def epilogue(s, ot, nh=1):
                    # out^T[h,q] = O^T[h,q] / den[q]. Reciprocal of the den
                    # row on DVE, partition-broadcast on idle GpSimd,
                    # multiply on DVE — nothing on the tensor engine. Last
                    # span runs in halves to shorten the kernel-final tail.
                    for e in range(nh):
                        w = 512 // nh
                        cs = slice(w * e, w * (e + 1))
                        rden = rdp.tile([1, 512], F32, tag="rd")
                        nc.vector.reciprocal(rden[:, 0:w], ot[H : H + 1, cs])
                        rb = rbp.tile([H, 512], F32, tag="rb")
                        nc.gpsimd.partition_broadcast(rb[:, 0:w], rden[:, 0:w])
                        ob = obp.tile([H, 512], F32, tag="ob")
                        nc.vector.scalar_tensor_tensor(
                            ob[:, 0:w], ot[0:H, cs], 1.0, rb[:, 0:w],
                            ALU.mult, ALU.mult,
                        )
                        c0 = 512 * s + w * e
                        nc.sync.dma_start(
                            out_d.ap()[:, c0 : c0 + w], ob[:, 0:w]
                        )

                kv_proj(0)
                q_proj(0)
                kv_proj(1)
                q_proj(1)
                ot1 = attention2(
                    0,
                    1,
                    {
                        0: [lambda: kv_proj(NSP)],
                        2: [lambda: kv_proj(NSP + 1)],
                        4: [lambda: kv_proj(2)],
                        6: [lambda: kv_proj(NSP + 2)],
                    },
                    lambda ot: epilogue(0, ot),
                )
                epilogue(1, ot1)
                q_proj(2)
                q_proj(3)
                ot3 = attention2(
                    2,
                    3,
                    {
                        0: [lambda: kv_proj(3)],
                        6: [lambda: kv_proj(NSP + 3)],
                    },
                    lambda ot: epilogue(2, ot),
                )
                epilogue(3, ot3, nh=2)

    nc.compile()
    return nc


def make_in_maps(x, Wk, Wq, Wv, T):
    """Per-core input dicts. x already [B, T, E] fp32 (np)."""
    import ml_dtypes

    wkv = np.concatenate([Wk, Wv], axis=1)  # [E, 128]
    wkv = np.ascontiguousarray(
        wkv.reshape(8, 128, 2 * H).transpose(1, 0, 2).reshape(128, 8 * 2 * H)
    )
    wqr = np.ascontiguousarray(
        Wq.reshape(8, 128, H).transpose(1, 0, 2).reshape(128, 8 * H)
    )
    in_maps = []
    NB = T // 256
    ch = np.arange(128)[:, None]
    col = np.arange(512)[None, :]
    tri = [(col - ch >= d).astype(np.float32) for d in (0, 128, 256, 384)]
    cge = (col - 0 * ch >= 256).astype(np.float32)
    ones = np.ones((128, 512), np.float32)
    zeros = np.zeros((128, 512), np.float32)
    mask_p = {
        0: np.stack(tri + [cge, cge, zeros, zeros]),
        1: np.stack(tri + [ones, ones, cge, cge]),
    }
    for core in range(NCORES):
        b, p = core // 2, core % 2
        blocks = list(range(p, NB, 2)) + list(range(1 - p, NB, 2))
        cols = np.concatenate(
            [np.arange(256 * blk, 256 * (blk + 1)) for blk in blocks]
        )
        xt = x[b].T[:, cols]  # [E, T]
        # [c*128+p, t*512+col] -> [p, (t, c, col)] so each per-partition DMA
        # run (one tile, all chunks) is contiguous in DRAM
        xt = np.ascontiguousarray(
            xt.reshape(8, 128, T // 512, 512).transpose(1, 2, 0, 3).reshape(128, -1)
        )
        in_maps.append(
            {
                "xt": xt.astype(ml_dtypes.bfloat16),
                "wkv": wkv.astype(ml_dtypes.bfloat16),
                "wq": wqr.astype(ml_dtypes.bfloat16),
                "masks": np.ascontiguousarray(
                    mask_p[p].transpose(1, 0, 2)
                ).astype(ml_dtypes.bfloat16),
            }
        )
    return in_maps


def gather_out(results, T):
    """results: list of per-core {name: array}. Returns [B, T, H]."""
    out = np.empty((B, T, H), np.float32)
    NB = T // 256
    for core in range(NCORES):
        b, p = core // 2, core % 2
        o = results[core]["out"]  # [H, T//2]
        own = list(range(p, NB, 2))
        for i, blk in enumerate(own):
            out[b, 256 * blk : 256 * (blk + 1), :] = o[
                :, 256 * i : 256 * (i + 1)
            ].T
    return out


_CACHE = {}


def kernel(x, Wk, Wq, Wv):
    x = np.asarray(x, np.float32)
    Wk = np.asarray(Wk, np.float32)
    Wq = np.asarray(Wq, np.float32)
    Wv = np.asarray(Wv, np.float32)
    T = x.shape[1]
    if T not in _CACHE:
        _CACHE[T] = build_program(T)
    nc = _CACHE[T]
    in_maps = make_in_maps(x, Wk, Wq, Wv, T)
    trace = os.environ.get("KERNEL_TRACE", "0") == "1"
    tdir = None
    if trace:
        tdir = os.environ.get("KERNEL_TRACE_DIR") or None
        if tdir:
            kernel.ncall = getattr(kernel, "ncall", -1) + 1
            tdir = os.path.join(tdir, f"call{kernel.ncall}")
            os.makedirs(tdir, exist_ok=True)
    res = bass_utils.run_bass_kernel_spmd(
        nc, in_maps, core_ids=list(range(NCORES)), trace=trace, tmpdir=tdir
    )
    kernel.exec_ns = res.exec_time_ns
    kernel.last_res = res
    return gather_out(res.results, T)


# revision 51
# speedup vs baseline: 1.1741x; 1.1741x over previous
"""Single-head causal attention on 8 TRN2 NeuronCores (Bass/Tile).

Problem: x[B=4,T=4096,E=1024] fp32; Wq/Wk/Wv [E,64]. out = softmax(causal(QK^T/8)) V.

Sharding: core i = (batch b=i//2, parity p=i%2). Each core computes the output
rows for the 256-token blocks of batch b with block index ≡ p (mod 2); one
uniform SPMD program, all per-core variation is input data.

v2 datapath (all-bf16 matmuls, 512-query spans):
  K^T,V^T projected packed in bf16 (PSUM fp32 acc over 8 E-chunks); V^T
  transposed to V-natural via PE in bf16. Q^T projected per 512-token span.
  Scores S^T[k,q] as [128,512] tiles; exp on ACT over paired 2-bank PSUM
  reads [128,1024] -> bf16 P (no max subtraction; |score*scale| <= ~3.6).
  Causal masks via (iota >= D) * P on DVE: own-parity tails use iota1=c-ch
  with constant D in {0,128,256,384}; other-parity tails use iota2=c with
  per-core D from dtab. P^T @ [V|1] accumulates O^T + softmax denominator in
  one PSUM group per span. Epilogue: reciprocal of den row, GpSimd
  partition-broadcast, DVE multiply; out stored transposed [H, T/2] (host
  gather transposes). KV projection of later tiles is interleaved between
  attention pairs to keep the PE busy while ACT drains.
"""

import os
import numpy as np

import concourse.bass as bass
import concourse.tile as tile
from concourse import bacc, bass_utils, mybir
from concourse.masks import make_identity

F32 = mybir.dt.float32
BF16 = mybir.dt.bfloat16
AF = mybir.ActivationFunctionType
ALU = mybir.AluOpType

B, T_FULL, E, H = 4, 4096, 1024, 64
NCORES = 8
SCALE = float(H) ** -0.5


def _patch_act_tables():
    """Make the act-table-load pass resolve both Exp and Ln to the combined
    natural_log_exp_and_others set (one table load instead of thrashing
    between exp_and_others and natural_log). Only narrows the pass's
    choices; table ids and runtime content are untouched."""
    import concourse.bacc as bacc_mod
    from concourse.hw_specs import get_activation_tables as _orig

    def patched(arch):
        tables = dict(_orig(arch))
        both = {AF.Exp, AF.Ln}
        if any(both <= fns for fns in tables.values()):
            for name, fns in tables.items():
                if not both <= fns:
                    tables[name] = fns - both
        return tables

    bacc_mod.get_activation_tables = patched


def build_program(T):
    _patch_act_tables()
    EC = E // 128            # 8 E-chunks
    NT = T // 512            # 8 x^T tiles (0-3 own tokens, 4-7 other)
    NSP = T // 1024          # 4 spans of 512 own queries
    NKT = T // 128           # 32 total 128-key tiles
    KO = NKT // 2            # 16 own k-tiles (kt col offset T//2 for other)

    nc = bacc.Bacc(
        "TRN2", target_bir_lowering=False, debug=False, num_devices=NCORES
    )
    # x^T pre-arranged by host as [p, tile, chunk, col] so each per-partition
    # DMA run is contiguous (8KB descriptors instead of 1KB).
    xt_d = nc.dram_tensor("xt", [128, (T // 512) * EC * 512], BF16, kind="ExternalInput")
    wkv_d = nc.dram_tensor("wkv", [128, EC * 2 * H], BF16, kind="ExternalInput")
    wq_d = nc.dram_tensor("wq", [128, EC * H], BF16, kind="ExternalInput")
    mask_d = nc.dram_tensor("masks", [128, 8, 512], BF16, kind="ExternalInput")
    out_d = nc.dram_tensor("out", [H, T // 2], F32, kind="ExternalOutput")

    with tile.TileContext(nc) as tc:
        with (
            tc.tile_pool(name="persist", bufs=1) as pp,
            tc.tile_pool(name="stage", bufs=3) as sp,
            tc.tile_pool(name="ppool", bufs=4) as ptp,
            tc.tile_pool(name="rdp", bufs=2) as rdp,
            tc.tile_pool(name="rbp", bufs=2) as rbp,
            tc.tile_pool(name="obp", bufs=2) as obp,
        ):
            # ---- persistent SBUF ----
            xt = [pp.tile([128, EC, 512], BF16, tag=f"xt{t}", name=f"xt{t}") for t in range(NT)]
            kt = pp.tile([64, T], BF16, tag="kt")
            vb = pp.tile([128, NKT, H + 1], BF16, tag="vb")
            qt = pp.tile([64, NSP, 512], BF16, tag="qt")
            wkv = pp.tile([128, EC, 2 * H], BF16, tag="wkv")
            wq = pp.tile([128, EC, H], BF16, tag="wq")
            masks = pp.tile([128, 8, 512], BF16, tag="masks")
            identb = pp.tile([128, 128], BF16, tag="identb")
            identf = pp.tile([H + 1, H + 1], F32, tag="identf")

            # ---- constants / small inputs ----
            nc.sync.dma_start(wkv, wkv_d.ap().rearrange("p (c m) -> p c m", c=EC))
            nc.sync.dma_start(wq, wq_d.ap().rearrange("p (c m) -> p c m", c=EC))
            make_identity(nc, identb)
            make_identity(nc, identf)
            nc.vector.memset(
                vb[:, :, H : H + 1].bitcast(mybir.dt.uint16), 0x3F80
            )

            # ---- stream x^T in program need order; first tile finer-grained ----
            xsrc = xt_d.ap().rearrange("p (n c t) -> p n c t", n=NT, c=EC)
            for c in range(0, 8, 2):
                nc.sync.dma_start(xt[0][:, c : c + 2, :], xsrc[:, 0, c : c + 2, :])
            for t in (1, NSP):
                nc.sync.dma_start(xt[t][:, 0:4, :], xsrc[:, t, 0:4, :])
                nc.sync.dma_start(xt[t][:, 4:8, :], xsrc[:, t, 4:8, :])
            nc.sync.dma_start(masks, mask_d.ap())
            for t in (NSP + 1, 2, NSP + 2, 3, NSP + 3):
                nc.sync.dma_start(xt[t][:, 0:4, :], xsrc[:, t, 0:4, :])
                nc.sync.dma_start(xt[t][:, 4:8, :], xsrc[:, t, 4:8, :])

            with (
                tc.tile_pool(name="spsum", bufs=2, space="PSUM") as ssp,
                tc.tile_pool(name="opsum", bufs=2, space="PSUM") as otp,
                tc.tile_pool(name="miscpsum", bufs=2, space="PSUM") as mp,
            ):
                def kv_proj(t):
                    kvfull = ssp.tile([128, 1024], F32, tag="s", name="kvfull")
                    acc = kvfull[:, 0:512]
                    for c in range(EC):
                        nc.tensor.matmul(
                            acc,
                            wkv[:, c, :],
                            xt[t][:, c, :],
                            start=(c == 0),
                            stop=(c == EC - 1),
                        )
                    kvs = sp.tile([128, 512], BF16, tag="kvs")
                    nc.vector.tensor_copy(kvs, acc)
                    nc.vector.tensor_copy(
                        kt[:, 512 * t : 512 * (t + 1)], kvs[0:64, :]
                    )
                    vg = mp.tile([128, 4, H], BF16, tag="vg")
                    for j in range(4):
                        nc.tensor.transpose(
                            vg[:, j, :],
                            kvs[64:128, 128 * j : 128 * (j + 1)],
                            identb[64:128, 64:128],
                        )
                    nc.vector.tensor_copy(vb[:, 4 * t : 4 * t + 4, 0:H], vg)

                def q_proj(s):
                    qfull = ssp.tile([128, 1024], F32, tag="s", name="qfull")
                    qacc = qfull[0:64, 0:512]
                    for c in range(EC):
                        nc.tensor.matmul(
                            qacc,
                            wq[:, c, :],
                            xt[s][:, c, :],
                            start=(c == 0),
                            stop=(c == EC - 1),
                        )
                    nc.vector.tensor_copy(qt[:, s, :], qacc)

                def span_pairs(s):
                    # k-tile list: (region_base_col, j, mask_col)
                    nk = 4 * s + 4
                    tiles = []
                    for j in range(nk):
                        m = (j - 4 * s) if j >= 4 * s else None
                        tiles.append((0, j, m))
                    for j in range(nk):
                        m = (4 + j - 4 * s) if j >= 4 * s else None
                        tiles.append((T // 2, j, m))
                    return [tiles[i : i + 2] for i in range(0, len(tiles), 2)]

                def do_pair(s, ot, pair, first, last):
                    st = ssp.tile([128, 1024], F32, tag="s", name="st")
                    for h, (base, j, m) in enumerate(pair):
                        kc = base + 128 * j
                        nc.tensor.matmul(
                            st[:, 512 * h : 512 * (h + 1)],
                            kt[:, kc : kc + 128],
                            qt[:, s, :],
                            start=True,
                            stop=True,
                        )
                    pt = ptp.tile([128, 1024], BF16, tag="pt")
                    nc.scalar.activation(pt, st, AF.Exp, scale=SCALE)
                    if pair[0][2] is not None or pair[1][2] is not None:
                        ptm = ptp.tile([128, 1024], BF16, tag="ptm")
                        for h, (base, j, m) in enumerate(pair):
                            nc.vector.scalar_tensor_tensor(
                                ptm[:, 512 * h : 512 * (h + 1)],
                                pt[:, 512 * h : 512 * (h + 1)],
                                1.0,
                                masks[:, m, :],
                                ALU.mult,
                                ALU.mult,
                            )
                        pt = ptm
                    for h, (base, j, m) in enumerate(pair):
                        vi = j if base == 0 else KO + j
                        nc.tensor.matmul(
                            ot,
                            vb[:, vi, :],
                            pt[:, 512 * h : 512 * (h + 1)],
                            start=(first and h == 0),
                            stop=(last and h == 1),
                        )

                def attention2(sa, sb, inject, epi_a):
                    """Interleave two spans' attention; span sa paced to
                    finish ~3 rounds early so its epilogue (epi_a) overlaps
                    span sb's ACT-bound tail. inject[i] thunks run at round
                    i."""
                    pa, pb = span_pairs(sa), span_pairs(sb)
                    na, nb = len(pa), len(pb)
                    ota = otp.tile([H + 1, 512], F32, tag="ot", name="ota")
                    otb = otp.tile([H + 1, 512], F32, tag="ot", name="otb")
                    ka = 0
                    done_a = False
                    for i in range(nb):
                        for fn in inject.get(i, []):
                            fn()
                        want = min(na, ((i + 1) * na) // max(1, nb - 3))
                        while ka < want:
                            do_pair(sa, ota, pa[ka], ka == 0, ka == na - 1)
                            ka += 1
                        if ka == na and not done_a:
                            done_a = True
                            epi_a(ota)
                        do_pair(sb, otb, pb[i], i == 0, i == nb - 1)
                    return otb
                def epilogue(s, ot):
                    # out^T[h,q] = O^T[h,q] / den[q]. 1/den = exp(-ln(den))
                    # on the scalar engine (cheap there; DVE reciprocal is
                    # 6.5ns/elem and was stalling span-critical mask ops),
                    # partition-broadcast on idle GpSimd, multiply on DVE.
                    lden = rdp.tile([1, 512], F32, tag="ld")
                    nc.scalar.activation(lden, ot[H : H + 1, :], AF.Ln)
                    rden = rdp.tile([1, 512], F32, tag="rd")
                    nc.scalar.activation(rden, lden, AF.Exp, scale=-1.0)
                    rb = rbp.tile([H, 512], F32, tag="rb")
                    nc.gpsimd.partition_broadcast(rb, rden)
                    ob = obp.tile([H, 512], F32, tag="ob")
                    nc.vector.scalar_tensor_tensor(
                        ob, ot[0:H, :], 1.0, rb, ALU.mult, ALU.mult
                    )
                    nc.sync.dma_start(
                        out_d.ap()[:, 512 * s : 512 * (s + 1)], ob
                    )

                kv_proj(0)
                q_proj(0)
                kv_proj(1)
                q_proj(1)
                ot1 = attention2(
                    0,
                    1,
                    {
                        0: [lambda: kv_proj(NSP)],
                        2: [lambda: kv_proj(NSP + 1)],
                        4: [lambda: kv_proj(2)],
                        6: [lambda: kv_proj(NSP + 2)],
                    },
                    lambda ot: epilogue(0, ot),
                )
                epilogue(1, ot1)
                q_proj(2)
                q_proj(3)
                ot3 = attention2(
                    2,
                    3,
                    {
                        0: [lambda: kv_proj(3)],
                        6: [lambda: kv_proj(NSP + 3)],
                    },
                    lambda ot: epilogue(2, ot),
                )
                epilogue(3, ot3)

    nc.compile()
    return nc


def make_in_maps(x, Wk, Wq, Wv, T):
    """Per-core input dicts. x already [B, T, E] fp32 (np)."""
    import ml_dtypes

    wkv = np.concatenate([Wk, Wv], axis=1)  # [E, 128]
    wkv = np.ascontiguousarray(
        wkv.reshape(8, 128, 2 * H).transpose(1, 0, 2).reshape(128, 8 * 2 * H)
    )
    wqr = np.ascontiguousarray(
        Wq.reshape(8, 128, H).transpose(1, 0, 2).reshape(128, 8 * H)
    )
    in_maps = []
    NB = T // 256
    ch = np.arange(128)[:, None]
    col = np.arange(512)[None, :]
    tri = [(col - ch >= d).astype(np.float32) for d in (0, 128, 256, 384)]
    cge = (col - 0 * ch >= 256).astype(np.float32)
    ones = np.ones((128, 512), np.float32)
    zeros = np.zeros((128, 512), np.float32)
    mask_p = {
        0: np.stack(tri + [cge, cge, zeros, zeros]),
        1: np.stack(tri + [ones, ones, cge, cge]),
    }
    for core in range(NCORES):
        b, p = core // 2, core % 2
        blocks = list(range(p, NB, 2)) + list(range(1 - p, NB, 2))
        cols = np.concatenate(
            [np.arange(256 * blk, 256 * (blk + 1)) for blk in blocks]
        )
        xt = x[b].T[:, cols]  # [E, T]
        # [c*128+p, t*512+col] -> [p, (t, c, col)] so each per-partition DMA
        # run (one tile, all chunks) is contiguous in DRAM
        xt = np.ascontiguousarray(
            xt.reshape(8, 128, T // 512, 512).transpose(1, 2, 0, 3).reshape(128, -1)
        )
        in_maps.append(
            {
                "xt": xt.astype(ml_dtypes.bfloat16),
                "wkv": wkv.astype(ml_dtypes.bfloat16),
                "wq": wqr.astype(ml_dtypes.bfloat16),
                "masks": np.ascontiguousarray(
                    mask_p[p].transpose(1, 0, 2)
                ).astype(ml_dtypes.bfloat16),
            }
        )
    return in_maps


def gather_out(results, T):
    """results: list of per-core {name: array}. Returns [B, T, H]."""
    out = np.empty((B, T, H), np.float32)
    NB = T // 256
    for core in range(NCORES):
        b, p = core // 2, core % 2
        o = results[core]["out"]  # [H, T//2]
        own = list(range(p, NB, 2))
        for i, blk in enumerate(own):
            out[b, 256 * blk : 256 * (blk + 1), :] = o[
                :, 256 * i : 256 * (i + 1)
            ].T
    return out


_CACHE = {}


def kernel(x, Wk, Wq, Wv):
    x = np.asarray(x, np.float32)
    Wk = np.asarray(Wk, np.float32)
    Wq = np.asarray(Wq, np.float32)
    Wv = np.asarray(Wv, np.float32)
    T = x.shape[1]
    if T not in _CACHE:
        _CACHE[T] = build_program(T)
    nc = _CACHE[T]
    in_maps = make_in_maps(x, Wk, Wq, Wv, T)
    trace = os.environ.get("KERNEL_TRACE", "0") == "1"
    tdir = None
    if trace:
        tdir = os.environ.get("KERNEL_TRACE_DIR") or None
        if tdir:
            kernel.ncall = getattr(kernel, "ncall", -1) + 1
            tdir = os.path.join(tdir, f"call{kernel.ncall}")
            os.makedirs(tdir, exist_ok=True)
    res = bass_utils.run_bass_kernel_spmd(
        nc, in_maps, core_ids=list(range(NCORES)), trace=trace, tmpdir=tdir
    )
    kernel.exec_ns = res.exec_time_ns
    kernel.last_res = res
    return gather_out(res.results, T)
